# revision 1
# baseline (speedup 1.0000x reference)
"""Embedding-similarity group merge on 8 Trainium2 NeuronCores.

Strategy
--------
The heavy part of the reference (Embeddings._fast_predict) is the blocked
cosine-similarity score computation V @ V.T (16384 x 16384 x 256 ~ 137 GFLOP).
The transitive group-merge that follows is inherently sequential and
path-dependent (final labels are NOT canonical connected-component ids), but
it only touches the ~20k above-threshold pairs, so it is cheap on host.

Device: bf16 matmul (full PE rate) producing a uint8 candidate mask
(sims_bf16 >= thr - EPS).  With both operands rounded to bf16,
|sims_bf16 - sims_fp32| <= 2^-8 + accumulation noise << EPS = 0.01, so the
mask is a guaranteed superset of the true fp32-threshold matches.

The reference only inspects j >= (i//B)*B + 1 (upper triangle plus a small
intra-batch band), so only j-tiles covering j >= 128*T are computed for
global i-tile T (~53% of the matrix).  The 128 i-tiles are dealt to cores
in an interleaved pattern (slot 2k -> tile 16k+c, slot 2k+1 -> 16k+15-c)
so every core runs the identical SPMD program: slot s starts at j-tile
JSTART[s] (4k for slot 2k, 4k+2 for slot 2k+1), which covers every core's
i-tile in that slot with at most one extra j-tile of slack.  Blocks are
emitted j-ascending so matmuls consume V.T progressively while it streams
in from HBM.

Thresholding splits across the Vector engine (tensor_scalar is_ge) and the
otherwise-idle Scalar engine (Sign(sims - thr), f32->u8 saturation maps
negatives to 0), one [128, 2048] op per 4-bank psum group.

Host: gathers candidate pairs, recomputes their sims exactly in fp32,
applies the reference's column mask, and replays the reference's
sequential batch/row merge to produce bit-identical group ids.
"""

import sys

if "/opt/trn_rl_repo" not in sys.path:
    sys.path.insert(0, "/opt/trn_rl_repo")

import numpy as np
import ml_dtypes

import concourse.bass as bass
import concourse.tile as tile
from concourse import bacc, mybir
from concourse.bass_utils import run_bass_kernel_spmd

N_CORES = 8
D = 256                     # embedding dim (2 chunks of 128 on partitions)
EPS = 0.01                  # bf16 guard band (worst-case bf16 error ~0.004)
I_TILE = 128                # psum partition tile (query rows per matmul)
J_TILE = 512                # matmul free-dim tile (one psum bank, fp32)
J_GROUP = 2                 # j-tiles per psum tile / compare / output DMA
SLOTS = 16                  # i-tiles per core

_BUILD_CACHE: dict = {}
LAST_EXEC_NS = None         # set when kernel() runs with TRACE=True
TRACE = False


def _itile_for_slot(c: int, s: int) -> int:
    """Global i-tile handled by core c in slot s (uniform-jstart interleave)."""
    k, r = divmod(s, 2)
    return 16 * k + (c if r == 0 else 15 - c)


def _jstart_for_slot(s: int) -> int:
    k, r = divmod(s, 2)
    return 4 * k + 2 * r


def _block_layout(n_jtiles: int):
    """Program-order output blocks, j-ascending: list of (slot, j0)."""
    blocks = []
    for s in range(SLOTS):
        j0 = _jstart_for_slot(s)
        while j0 < n_jtiles:
            blocks.append((s, j0))
            j0 += J_GROUP
    blocks.sort(key=lambda b: (b[1], b[0]))
    return blocks


def _ensure_ntff_hook():
    """Register the axon NTFF-profile hook (test/trace path only).

    The agent image's ``antenv`` lacks ``axon_hooks``, so ``trn_boot.boot``
    silently skips hook registration and ``bass_utils`` would crash on the
    import. Seed ``sys.modules['antenv.axon_hooks']`` with a stub wired to
    the ctypes hook so ``trace=True`` yields real NTFF profiles."""
    import types
    if "antenv.axon_hooks" in sys.modules:
        return
    try:
        from trn_agent_boot.trn_boot import _ntff_profile_via_ctypes
        hook = _ntff_profile_via_ctypes("/opt/axon/libaxon_pjrt.so")
    except Exception:
        hook = None
    mod = types.ModuleType("antenv.axon_hooks")
    mod._HOOK = hook
    mod.get_axon_ntff_profile_hook = lambda: mod._HOOK
    mod.set_axon_ntff_profile_hook = lambda h: setattr(mod, "_HOOK", h)
    sys.modules["antenv.axon_hooks"] = mod


def _build_program(n_cols: int, thr_dev: float) -> bass.Bass:
    """One SPMD program, identical across cores; per-core behaviour comes
    only from the vq input (each core's 4 quads of query columns).

    Inputs (per core):
      vt [2, 128, n_cols] bf16 -- V.T split into two 128-row d-chunks
      vq [2, 128, 2048] bf16   -- this core's 4 quads (4*512 query columns)
    Output:
      out [n_blocks, 128, J_GROUP*J_TILE] u8 -- candidate mask blocks
    """
    n_jtiles = n_cols // J_TILE
    blocks = _block_layout(n_jtiles)
    rows = SLOTS * I_TILE

    nc = bacc.Bacc(None, target_bir_lowering=False)
    vt_d = nc.declare_dram_parameter("vt", [2, 128, n_cols], mybir.dt.bfloat16, isOutput=False)
    vq_d = nc.declare_dram_parameter("vq", [2, 128, rows], mybir.dt.bfloat16, isOutput=False)
    out_d = nc.declare_dram_parameter(
        "out", [len(blocks), I_TILE, J_GROUP * J_TILE], mybir.dt.uint8, isOutput=True)

    gw = J_GROUP * J_TILE
    with tile.TileContext(nc) as tc:
        with (
            tc.tile_pool(name="vt", bufs=1) as vt_pool,
            tc.tile_pool(name="vq", bufs=1) as vq_pool,
            tc.tile_pool(name="psum", bufs=4, space="PSUM") as psum_pool,
            tc.tile_pool(name="stage", bufs=6) as stage_pool,
        ):
            vt_sb = vt_pool.tile([128, 2, n_cols], mybir.dt.bfloat16)
            vq_sb = vq_pool.tile([128, 2, rows], mybir.dt.bfloat16)
            bias_t = vq_pool.tile([128, 1], mybir.dt.float32)
            nc.vector.memset(bias_t, -thr_dev)
            # Interleave vq pieces with vt parts so the first matmuls gate on
            # ~1MB of DMA, not the full 9MB, and vt streams ahead of the
            # j-ascending consumption order.  Inputs own the HWDGE queues
            # (outputs go via SWDGE) so the stream is never starved.
            part_edges = [0, 512, 1024, 2048]
            while part_edges[-1] < n_cols:
                part_edges.append(min(n_cols, part_edges[-1] + 2048))
            vq_parts = 4
            vqw = rows // vq_parts
            for p in range(max(len(part_edges) - 1, vq_parts)):
                for c in range(2):
                    eng = nc.sync
                    if p < vq_parts:
                        eng.dma_start(
                            out=vq_sb[:, c, p * vqw:(p + 1) * vqw],
                            in_=vq_d[c, :, p * vqw:(p + 1) * vqw])
                    if p < len(part_edges) - 1:
                        lo, hi = part_edges[p], part_edges[p + 1]
                        eng.dma_start(
                            out=vt_sb[:, c, lo:hi],
                            in_=vt_d[c, :, lo:hi])

            for k, (s, j0) in enumerate(blocks):
                ts = slice(s * I_TILE, (s + 1) * I_TILE)
                ps = psum_pool.tile([128, gw], mybir.dt.float32)
                for c in range(2):
                    for jj in range(J_GROUP):
                        js = slice((j0 + jj) * J_TILE, (j0 + jj + 1) * J_TILE)
                        nc.tensor.matmul(
                            ps[:, jj * J_TILE:(jj + 1) * J_TILE],
                            lhsT=vq_sb[:, c, ts], rhs=vt_sb[:, c, js],
                            start=(c == 0), stop=(c == 1),
                        )
                stage = stage_pool.tile([128, gw], mybir.dt.uint8)
                if k % 2 == 0:
                    nc.vector.tensor_scalar(
                        stage, ps, thr_dev, None, mybir.AluOpType.is_ge)
                else:
                    # Sign(sims - thr): +1 above threshold; 0/255 otherwise
                    # (f32->u8 of -1 may wrap). Host treats ==1 as candidate.
                    nc.scalar.activation(
                        stage, ps, mybir.ActivationFunctionType.Sign,
                        bias=bias_t)
                # Early blocks overlap the input stream: keep them off the
                # HWDGE queues (SWDGE). Once the input has landed, HWDGE is
                # free and drains the later (larger) share of the output.
                if k < len(blocks) // 4:
                    nc.gpsimd.dma_start(out=out_d[k], in_=stage)
                else:
                    nc.sync.dma_start(out=out_d[k], in_=stage)
    nc.finalize()
    return nc


def _device_candidate_edges(V32: np.ndarray, thr: float):
    """Run the SPMD kernel on 8 cores; return candidate pairs (ci, cj) with
    sims_bf16 >= thr - EPS, restricted to the computed upper-triangle blocks
    (a superset of every pair the reference's column mask admits)."""
    global LAST_EXEC_NS
    n = V32.shape[0]
    thr_dev = float(thr) - EPS

    key = (n, round(thr_dev, 9))
    if key not in _BUILD_CACHE:
        _BUILD_CACHE[key] = _build_program(n, thr_dev)
    nc = _BUILD_CACHE[key]

    vt16 = np.ascontiguousarray(V32.T.reshape(2, 128, n).astype(ml_dtypes.bfloat16))
    in_maps = []
    for c in range(N_CORES):
        cols = np.concatenate([
            np.arange(I_TILE * _itile_for_slot(c, s),
                      I_TILE * (_itile_for_slot(c, s) + 1))
            for s in range(SLOTS)])
        vq16 = np.ascontiguousarray(vt16[:, :, cols])
        in_maps.append({"vt": vt16, "vq": vq16})

    if TRACE:
        _ensure_ntff_hook()
    res = run_bass_kernel_spmd(
        nc, in_maps, core_ids=list(range(N_CORES)), trace=TRACE)
    if TRACE:
        LAST_EXEC_NS = res.exec_time_ns

    blocks = _block_layout(n // J_TILE)
    s_arr = np.array([b[0] for b in blocks], dtype=np.int64)
    j0_arr = np.array([b[1] for b in blocks], dtype=np.int64)
    ci_all, cj_all = [], []
    for c in range(N_CORES):
        o = res.results[c]["out"]  # [n_blocks, 128, gw]
        bi, bp, bq = np.nonzero(o == 1)
        if bi.size == 0:
            continue
        t_arr = np.array([_itile_for_slot(c, s) for s in range(SLOTS)],
                         dtype=np.int64)[s_arr]
        ci_all.append(I_TILE * t_arr[bi] + bp)
        cj_all.append(J_TILE * j0_arr[bi] + bq)
    if not ci_all:
        return (np.zeros(0, np.int64), np.zeros(0, np.int64))
    return np.concatenate(ci_all), np.concatenate(cj_all)


def _exact_edges(V32, ci, cj, thr, B):
    """From candidate pairs, produce exact reference edges:
    fp32 sims >= thr and j >= (i//B)*B + 1.  Returns (ci, cj)."""
    keep = cj >= (ci // B) * B + 1
    ci, cj = ci[keep], cj[keep]
    if ci.size:
        sims = np.einsum("ij,ij->i", V32[ci], V32[cj])
        keep = sims >= np.float32(thr)
        ci, cj = ci[keep], cj[keep]
    return ci, cj


def _merge_replay(g, ci, cj, B):
    """Faithful replay of the reference's sequential merge.

    Per batch: the matched sets are frozen at batch start (with the
    g_i0 != g_j filter evaluated on batch-start group ids), then rows are
    processed sequentially; each row i merges every row whose CURRENT group
    id appears among the CURRENT group ids of its matched j's into i's
    CURRENT group."""
    n = g.shape[0]
    if ci.size == 0:
        return g
    order = np.argsort(ci, kind="stable")
    ci, cj = ci[order], cj[order]
    row_ids, row_starts = np.unique(ci, return_index=True)
    row_ends = np.append(row_starts[1:], ci.size)
    row_j = {int(i): cj[s:e] for i, s, e in zip(row_ids, row_starts, row_ends)}

    flag = np.zeros(max(n, int(g.max()) + 1), dtype=bool)
    for b in np.unique(row_ids // B):
        bs = int(b) * B
        g0 = g.copy()
        frozen = []
        for i in range(bs, bs + B):
            J = row_j.get(i)
            if J is None:
                continue
            J = J[g0[J] != g0[i]]
            if J.size:
                frozen.append((i, J))
        for i, J in frozen:
            mg = np.unique(g[J])
            flag[mg] = True
            sel = flag[g]
            g[sel] = g[i]
            flag[mg] = False
    return g


def kernel(V, group_ids, cos_threshold, batch_size):
    V32 = np.ascontiguousarray(np.asarray(V, dtype=np.float32))
    g = np.asarray(group_ids, dtype=np.int32).copy()
    thr = float(np.asarray(cos_threshold).reshape(-1)[0])
    B = int(np.asarray(batch_size))

    ci, cj = _device_candidate_edges(V32, thr)
    ci, cj = _exact_edges(V32, ci, cj, thr, B)
    g = _merge_replay(g, ci, cj, B)
    return g.astype(np.int32)



# revision 4
# speedup vs baseline: 1.0647x; 1.0647x over previous
"""Embedding-similarity group merge on 8 Trainium2 NeuronCores.

Strategy (v2: fp8 DoubleRow + split psum scan)
----------------------------------------------
The heavy part of the reference (Embeddings._fast_predict) is the blocked
cosine-similarity score computation V @ V.T (16384 x 16384 x 256).  The
transitive group-merge that follows is sequential and path-dependent but only
touches the ~10k above-threshold pairs, so it is replayed exactly on host.

Device work per core (SPMD, identical program; per-core behaviour comes only
from the vq input = that core's 16 interleaved query i-tiles):

* Matmul in fp8e4 with perf_mode=DoubleRow: the PE array virtualizes to
  128x256, so the full D=256 contraction happens in ONE matmul per
  [128 x 512] output tile at 2 fp8 elements/cycle -- 2x the bf16 rate.
  With both operands rounded to fp8e4, |sims_fp8 - sims_fp32| < EPS = 0.013
  (validated empirically; error std ~1e-3, max ~7e-3), so thresholding at
  thr - EPS yields a guaranteed superset of the true fp32 matches.
* The psum scan (the roofline wall: only Vector and Scalar can read PSUM,
  both at 1 element/cycle/lane) is split per 4-bank psum supertile between:
    - Scalar: Sign(sims - thr') -> u8 mask [128, 2048] -> DMA (exact cols)
    - Vector: segmented reduce_max -> [128, 16] fp32 (128-col segments),
      host re-expands flagged segments (tiny DMA instead of 256KB)
  Units are assigned greedily to balance the two engines' cycle counts.

Host: expands/gathers candidate pairs, recomputes their sims exactly in
fp32, applies the reference's column mask, and replays the reference's
sequential batch/row merge to produce bit-identical group ids.
"""

import sys

if "/opt/trn_rl_repo" not in sys.path:
    sys.path.insert(0, "/opt/trn_rl_repo")

import numpy as np
import ml_dtypes

import concourse.bass as bass
import concourse.tile as tile
from concourse import bacc, mybir
from concourse.bass_utils import run_bass_kernel_spmd

N_CORES = 8
D = 256                     # embedding dim (2 chunks of 128 on partitions)
EPS = 0.030                 # fp8e4 guard band (measured max err 0.023 @67M pairs)
I_TILE = 128                # psum partition tile (query rows per matmul)
J_TILE = 512                # matmul free-dim tile (one psum bank, fp32)
WAVE = 4                    # j-tiles per psum supertile (4 banks)
SEG = 128                   # reduce-max segment width (cols)
SLOTS = 16                  # i-tiles per core

_BUILD_CACHE: dict = {}
LAST_EXEC_NS = None         # set when kernel() runs with TRACE=True
TRACE = False


def _itile_for_slot(c: int, s: int) -> int:
    """Global i-tile handled by core c in slot s (uniform-jstart interleave)."""
    k, r = divmod(s, 2)
    return 16 * k + (c if r == 0 else 15 - c)


def _jstart_for_slot(s: int) -> int:
    k, r = divmod(s, 2)
    return 4 * k + 2 * r


def _unit_layout(n_jtiles: int):
    """Program-order scan units, wave-ascending: (slot, jt0, n_jt)."""
    units = []
    n_waves = n_jtiles // WAVE
    for w in range(n_waves):
        for s in range(SLOTS):
            j0 = _jstart_for_slot(s)
            w0 = j0 // WAVE
            if w < w0:
                continue
            if w == w0:
                units.append((s, j0, (w0 + 1) * WAVE - j0))
            else:
                units.append((s, w * WAVE, WAVE))
    return units


def _assign_engines(units):
    """Greedy balance of scan cost: vector reduce vs scalar mask."""
    kinds = []
    v_acc = s_acc = 0.0
    for (_s, _j0, n_jt) in units:
        fd = n_jt * J_TILE
        v_cost = (120.0 + fd) / 0.96
        s_cost = (352.0 + fd) / 1.2
        if v_acc + v_cost <= s_acc + s_cost:
            kinds.append("max")
            v_acc += v_cost
        else:
            kinds.append("mask")
            s_acc += s_cost
    return kinds


def _ensure_ntff_hook():
    """Register the axon NTFF-profile hook (test/trace path only).

    The agent image's ``antenv`` lacks ``axon_hooks``, so ``trn_boot.boot``
    silently skips hook registration and ``bass_utils`` would crash on the
    import. Seed ``sys.modules['antenv.axon_hooks']`` with a stub wired to
    the ctypes hook so ``trace=True`` yields real NTFF profiles."""
    import types
    if "antenv.axon_hooks" in sys.modules:
        return
    try:
        from trn_agent_boot.trn_boot import _ntff_profile_via_ctypes
        hook = _ntff_profile_via_ctypes("/opt/axon/libaxon_pjrt.so")
    except Exception:
        hook = None
    mod = types.ModuleType("antenv.axon_hooks")
    mod._HOOK = hook
    mod.get_axon_ntff_profile_hook = lambda: mod._HOOK
    mod.set_axon_ntff_profile_hook = lambda h: setattr(mod, "_HOOK", h)
    sys.modules["antenv.axon_hooks"] = mod


def _build_program(n_cols: int, thr_dev: float) -> bass.Bass:
    """One SPMD program, identical across cores.

    Inputs (per core):
      vt [2, 128, n_cols] fp8e4 -- V.T split into two 128-row d-chunks
      vq [2, 128, 2048] fp8e4   -- this core's 16 i-tiles of query columns
    Outputs:
      mask [n_mask, 128, WAVE*J_TILE] u8    -- scalar-engine candidate masks
      mx   [n_max, 128, WAVE*J_TILE//SEG] f32 -- vector-engine segment maxes
    """
    n_jtiles = n_cols // J_TILE
    units = _unit_layout(n_jtiles)
    kinds = _assign_engines(units)
    rows = SLOTS * I_TILE
    gw = WAVE * J_TILE
    nseg = gw // SEG
    n_mask = sum(1 for k in kinds if k == "mask")
    n_max = len(kinds) - n_mask

    nc = bacc.Bacc(None, target_bir_lowering=False)
    vt_d = nc.declare_dram_parameter("vt", [2, 128, n_cols], mybir.dt.float8e4, isOutput=False)
    vq_d = nc.declare_dram_parameter("vq", [2, 128, rows], mybir.dt.float8e4, isOutput=False)
    mask_d = nc.declare_dram_parameter(
        "mask", [max(n_mask, 1), I_TILE, gw], mybir.dt.uint8, isOutput=True)
    mx_d = nc.declare_dram_parameter(
        "mx", [max(n_max, 1), I_TILE, nseg], mybir.dt.float32, isOutput=True)

    with tile.TileContext(nc) as tc:
        with (
            tc.tile_pool(name="vt", bufs=1) as vt_pool,
            tc.tile_pool(name="vq", bufs=1) as vq_pool,
            tc.tile_pool(name="psum", bufs=2, space="PSUM") as psum_pool,
            tc.tile_pool(name="stage", bufs=6) as stage_pool,
            tc.tile_pool(name="mxs", bufs=6) as mx_pool,
        ):
            vt_sb = vt_pool.tile([128, 2, n_cols], mybir.dt.float8e4)
            vq_sb = vq_pool.tile([128, 2, rows], mybir.dt.float8e4)
            bias_t = vq_pool.tile([128, 1], mybir.dt.float32)
            nc.vector.memset(bias_t, -thr_dev)
            # Interleave vq pieces with vt parts so the first matmuls gate on
            # a small DMA prefix and vt streams ahead of the wave-ascending
            # consumption order.  Inputs own the HWDGE queues (early outputs
            # go via SWDGE) so the stream is never starved.
            part_edges = [0, 512, 1024, 2048]
            while part_edges[-1] < n_cols:
                part_edges.append(min(n_cols, part_edges[-1] + 2048))
            vq_parts = 4
            vqw = rows // vq_parts
            for p in range(max(len(part_edges) - 1, vq_parts)):
                for c in range(2):
                    eng = nc.sync
                    if p < vq_parts:
                        eng.dma_start(
                            out=vq_sb[:, c, p * vqw:(p + 1) * vqw],
                            in_=vq_d[c, :, p * vqw:(p + 1) * vqw])
                    if p < len(part_edges) - 1:
                        lo, hi = part_edges[p], part_edges[p + 1]
                        eng.dma_start(
                            out=vt_sb[:, c, lo:hi],
                            in_=vt_d[c, :, lo:hi])

            i_mask = i_max = 0
            for k, ((s, jt0, n_jt), kind) in enumerate(zip(units, kinds)):
                fd = n_jt * J_TILE
                ts = slice(s * I_TILE, (s + 1) * I_TILE)
                ps = psum_pool.tile([128, nseg, SEG], mybir.dt.float32)
                for jj in range(n_jt):
                    js = slice((jt0 + jj) * J_TILE, (jt0 + jj + 1) * J_TILE)
                    nc.tensor.matmul(
                        ps[:, jj * (J_TILE // SEG):(jj + 1) * (J_TILE // SEG), :],
                        lhsT=vq_sb[:, :, ts], rhs=vt_sb[:, :, js],
                        start=True, stop=True,
                        perf_mode=mybir.MatmulPerfMode.DoubleRow,
                    )
                nsg = fd // SEG
                if kind == "mask":
                    stage = stage_pool.tile([128, nseg, SEG], mybir.dt.uint8)
                    # Sign(sims - thr'): +1 above threshold; 0/255 otherwise
                    # (f32->u8 of -1 may wrap). Host treats ==1 as candidate.
                    nc.scalar.activation(
                        stage[:, :nsg, :], ps[:, :nsg, :],
                        mybir.ActivationFunctionType.Sign, bias=bias_t)
                    if k < len(units) // 4:
                        nc.gpsimd.dma_start(
                            out=mask_d[i_mask, :, :fd],
                            in_=stage[:, :nsg, :])
                    else:
                        nc.sync.dma_start(
                            out=mask_d[i_mask, :, :fd],
                            in_=stage[:, :nsg, :])
                    i_mask += 1
                else:
                    mxt = mx_pool.tile([128, nseg], mybir.dt.float32)
                    nc.vector.tensor_reduce(
                        mxt[:, :nsg], ps[:, :nsg, :],
                        axis=mybir.AxisListType.X, op=mybir.AluOpType.max)
                    if k < len(units) // 4:
                        nc.gpsimd.dma_start(out=mx_d[i_max, :, :nsg], in_=mxt[:, :nsg])
                    else:
                        nc.sync.dma_start(out=mx_d[i_max, :, :nsg], in_=mxt[:, :nsg])
                    i_max += 1
    nc.finalize()
    return nc


def _device_candidate_edges(V32: np.ndarray, thr: float):
    """Run the SPMD kernel on 8 cores; return candidate pairs (ci, cj) with
    sims_fp8 >= thr - EPS, restricted to the computed upper-triangle blocks
    (a superset of every pair the reference's column mask admits).  Vector
    (reduce-max) units contribute whole 128-col segments per flagged row."""
    global LAST_EXEC_NS
    n = V32.shape[0]
    thr_dev = float(thr) - EPS

    key = (n, round(thr_dev, 9))
    if key not in _BUILD_CACHE:
        _BUILD_CACHE[key] = _build_program(n, thr_dev)
    nc = _BUILD_CACHE[key]

    vt8 = np.ascontiguousarray(
        V32.T.reshape(2, 128, n).astype(ml_dtypes.float8_e4m3))
    in_maps = []
    for c in range(N_CORES):
        cols = np.concatenate([
            np.arange(I_TILE * _itile_for_slot(c, s),
                      I_TILE * (_itile_for_slot(c, s) + 1))
            for s in range(SLOTS)])
        vq8 = np.ascontiguousarray(vt8[:, :, cols])
        in_maps.append({"vt": vt8, "vq": vq8})

    if TRACE:
        _ensure_ntff_hook()
    res = run_bass_kernel_spmd(
        nc, in_maps, core_ids=list(range(N_CORES)), trace=TRACE)
    if TRACE:
        LAST_EXEC_NS = res.exec_time_ns

    units = _unit_layout(n // J_TILE)
    kinds = _assign_engines(units)
    ci_all, cj_all = [], []
    for c in range(N_CORES):
        o_mask = res.results[c]["mask"]  # [n_mask, 128, gw]
        o_mx = res.results[c]["mx"]      # [n_max, 128, nseg]
        t_for_s = np.array([_itile_for_slot(c, s) for s in range(SLOTS)],
                           dtype=np.int64)
        i_mask = i_max = 0
        for (s, jt0, n_jt), kind in zip(units, kinds):
            fd = n_jt * J_TILE
            base_i = I_TILE * t_for_s[s]
            base_j = J_TILE * jt0
            if kind == "mask":
                o = o_mask[i_mask][:, :fd]
                bp, bq = np.nonzero(o == 1)
                if bp.size:
                    ci_all.append(base_i + bp)
                    cj_all.append(base_j + bq)
                i_mask += 1
            else:
                nsg = fd // SEG
                m = o_mx[i_max][:, :nsg]
                bp, bs = np.nonzero(m >= thr_dev)
                if bp.size:
                    # expand each flagged segment to its SEG columns
                    ci_all.append(np.repeat(base_i + bp, SEG))
                    cj_all.append(
                        (base_j + bs[:, None] * SEG
                         + np.arange(SEG)[None, :]).reshape(-1))
                i_max += 1
    if not ci_all:
        return (np.zeros(0, np.int64), np.zeros(0, np.int64))
    return np.concatenate(ci_all), np.concatenate(cj_all)


def _exact_edges(V32, ci, cj, thr, B):
    """From candidate pairs, produce exact reference edges:
    fp32 sims >= thr and j >= (i//B)*B + 1.  Returns (ci, cj)."""
    keep = cj >= (ci // B) * B + 1
    ci, cj = ci[keep], cj[keep]
    if ci.size:
        sims = np.empty(ci.size, np.float32)
        CH = 1 << 19
        for lo in range(0, ci.size, CH):
            hi = min(lo + CH, ci.size)
            sims[lo:hi] = np.einsum(
                "ij,ij->i", V32[ci[lo:hi]], V32[cj[lo:hi]])
        keep = sims >= np.float32(thr)
        ci, cj = ci[keep], cj[keep]
    return ci, cj


def _merge_replay(g, ci, cj, B):
    """Faithful replay of the reference's sequential merge.

    Per batch: the matched sets are frozen at batch start (with the
    g_i0 != g_j filter evaluated on batch-start group ids), then rows are
    processed sequentially; each row i merges every row whose CURRENT group
    id appears among the CURRENT group ids of its matched j's into i's
    CURRENT group."""
    n = g.shape[0]
    if ci.size == 0:
        return g
    order = np.argsort(ci, kind="stable")
    ci, cj = ci[order], cj[order]
    row_ids, row_starts = np.unique(ci, return_index=True)
    row_ends = np.append(row_starts[1:], ci.size)
    row_j = {int(i): cj[s:e] for i, s, e in zip(row_ids, row_starts, row_ends)}

    flag = np.zeros(max(n, int(g.max()) + 1), dtype=bool)
    for b in np.unique(row_ids // B):
        bs = int(b) * B
        g0 = g.copy()
        frozen = []
        for i in range(bs, bs + B):
            J = row_j.get(i)
            if J is None:
                continue
            J = J[g0[J] != g0[i]]
            if J.size:
                frozen.append((i, J))
        for i, J in frozen:
            mg = np.unique(g[J])
            flag[mg] = True
            sel = flag[g]
            g[sel] = g[i]
            flag[mg] = False
    return g


def kernel(V, group_ids, cos_threshold, batch_size):
    V32 = np.ascontiguousarray(np.asarray(V, dtype=np.float32))
    g = np.asarray(group_ids, dtype=np.int32).copy()
    thr = float(np.asarray(cos_threshold).reshape(-1)[0])
    B = int(np.asarray(batch_size))

    ci, cj = _device_candidate_edges(V32, thr)
    ci, cj = _exact_edges(V32, ci, cj, thr, B)
    g = _merge_replay(g, ci, cj, B)
    return g.astype(np.int32)


# revision 7
# speedup vs baseline: 1.1162x; 1.0483x over previous
"""Embedding-similarity group merge on 8 Trainium2 NeuronCores.

Strategy (v2: fp8 DoubleRow + split psum scan)
----------------------------------------------
The heavy part of the reference (Embeddings._fast_predict) is the blocked
cosine-similarity score computation V @ V.T (16384 x 16384 x 256).  The
transitive group-merge that follows is sequential and path-dependent but only
touches the ~10k above-threshold pairs, so it is replayed exactly on host.

Device work per core (SPMD, identical program; per-core behaviour comes only
from the vq input = that core's 16 interleaved query i-tiles):

* Matmul in fp8e4 with perf_mode=DoubleRow: the PE array virtualizes to
  128x256, so the full D=256 contraction happens in ONE matmul per
  [128 x 512] output tile at 2 fp8 elements/cycle -- 2x the bf16 rate.
  With both operands rounded to fp8e4, |sims_fp8 - sims_fp32| < EPS = 0.013
  (validated empirically; error std ~1e-3, max ~7e-3), so thresholding at
  thr - EPS yields a guaranteed superset of the true fp32 matches.
* The psum scan (the roofline wall: only Vector and Scalar can read PSUM,
  both at 1 element/cycle/lane) is split per 4-bank psum supertile between:
    - Scalar: Sign(sims - thr') -> u8 mask [128, 2048] -> DMA (exact cols)
    - Vector: segmented reduce_max -> [128, 16] fp32 (128-col segments),
      host re-expands flagged segments (tiny DMA instead of 256KB)
  Units are assigned greedily to balance the two engines' cycle counts.

Host: expands/gathers candidate pairs, recomputes their sims exactly in
fp32, applies the reference's column mask, and replays the reference's
sequential batch/row merge to produce bit-identical group ids.
"""

import sys

if "/opt/trn_rl_repo" not in sys.path:
    sys.path.insert(0, "/opt/trn_rl_repo")

import numpy as np
import ml_dtypes

import concourse.bass as bass
import concourse.tile as tile
from concourse import bacc, mybir
from concourse.bass_utils import run_bass_kernel_spmd

N_CORES = 8
D = 256                     # embedding dim (2 chunks of 128 on partitions)
EPS = 0.030                 # fp8e4 guard band (measured max err 0.023 @67M pairs)
I_TILE = 128                # psum partition tile (query rows per matmul)
J_TILE = 512                # matmul free-dim tile (one psum bank, fp32)
WAVE = 2                    # j-tiles per psum unit (2 banks; 4 units in flight)
SEG = 128                   # reduce-max segment width (cols)
SLOTS = 16                  # i-tiles per core

_BUILD_CACHE: dict = {}
LAST_EXEC_NS = None         # set when kernel() runs with TRACE=True
TRACE = False


def _itile_for_slot(c: int, s: int) -> int:
    """Global i-tile handled by core c in slot s (uniform-jstart interleave)."""
    k, r = divmod(s, 2)
    return 16 * k + (c if r == 0 else 15 - c)


def _jstart_for_slot(s: int) -> int:
    k, r = divmod(s, 2)
    return 4 * k + 2 * r


def _unit_layout(n_jtiles: int):
    """Program-order scan units, wave-ascending: (slot, jt0, n_jt)."""
    units = []
    n_waves = n_jtiles // WAVE
    for w in range(n_waves):
        for s in range(SLOTS):
            j0 = _jstart_for_slot(s)
            w0 = j0 // WAVE
            if w < w0:
                continue
            if w == w0:
                units.append((s, j0, (w0 + 1) * WAVE - j0))
            else:
                units.append((s, w * WAVE, WAVE))
    return units


def _assign_engines(units):
    """Greedy balance of scan cost: vector reduce vs scalar mask."""
    kinds = []
    v_acc = s_acc = 0.0
    for (_s, _j0, n_jt) in units:
        fd = n_jt * J_TILE
        v_cost = (120.0 + fd) / 0.96
        s_cost = (352.0 + fd) / 1.2
        if v_acc + v_cost <= s_acc + s_cost:
            kinds.append("max")
            v_acc += v_cost
        else:
            kinds.append("mask")
            s_acc += s_cost
    return kinds


def _ensure_ntff_hook():
    """Register the axon NTFF-profile hook (test/trace path only).

    The agent image's ``antenv`` lacks ``axon_hooks``, so ``trn_boot.boot``
    silently skips hook registration and ``bass_utils`` would crash on the
    import. Seed ``sys.modules['antenv.axon_hooks']`` with a stub wired to
    the ctypes hook so ``trace=True`` yields real NTFF profiles."""
    import types
    if "antenv.axon_hooks" in sys.modules:
        return
    try:
        from trn_agent_boot.trn_boot import _ntff_profile_via_ctypes
        hook = _ntff_profile_via_ctypes("/opt/axon/libaxon_pjrt.so")
    except Exception:
        hook = None
    mod = types.ModuleType("antenv.axon_hooks")
    mod._HOOK = hook
    mod.get_axon_ntff_profile_hook = lambda: mod._HOOK
    mod.set_axon_ntff_profile_hook = lambda h: setattr(mod, "_HOOK", h)
    sys.modules["antenv.axon_hooks"] = mod


def _build_program(n_cols: int, thr_dev: float) -> bass.Bass:
    """One SPMD program, identical across cores.

    Inputs (per core):
      vt [2, 128, n_cols] fp8e4 -- V.T split into two 128-row d-chunks
      vq [2, 128, 2048] fp8e4   -- this core's 16 i-tiles of query columns
    Outputs:
      mask [n_mask, 128, WAVE*J_TILE] u8    -- scalar-engine candidate masks
      mx   [n_max, 128, WAVE*J_TILE//SEG] f32 -- vector-engine segment maxes
    """
    n_jtiles = n_cols // J_TILE
    units = _unit_layout(n_jtiles)
    kinds = _assign_engines(units)
    rows = SLOTS * I_TILE
    gw = WAVE * J_TILE
    nseg = gw // SEG
    n_mask = sum(1 for k in kinds if k == "mask")
    n_max = len(kinds) - n_mask

    nc = bacc.Bacc(None, target_bir_lowering=False)
    vt_d = nc.declare_dram_parameter("vt", [2, 128, n_cols], mybir.dt.float8e4, isOutput=False)
    vq_d = nc.declare_dram_parameter("vq", [2, 128, rows], mybir.dt.float8e4, isOutput=False)
    mask_d = nc.declare_dram_parameter(
        "mask", [max(n_mask, 1), I_TILE, gw], mybir.dt.uint8, isOutput=True)
    mx_d = nc.declare_dram_parameter(
        "mx", [max(n_max, 1), I_TILE, nseg], mybir.dt.float32, isOutput=True)

    with tile.TileContext(nc) as tc:
        with (
            tc.tile_pool(name="vt", bufs=1) as vt_pool,
            tc.tile_pool(name="vq", bufs=1) as vq_pool,
            tc.tile_pool(name="psum", bufs=4, space="PSUM") as psum_pool,
            tc.tile_pool(name="stage", bufs=6) as stage_pool,
            tc.tile_pool(name="mxs", bufs=6) as mx_pool,
        ):
            vt_sb = vt_pool.tile([128, 2, n_cols], mybir.dt.float8e4)
            vq_sb = vq_pool.tile([128, 2, rows], mybir.dt.float8e4)
            bias_t = vq_pool.tile([128, 1], mybir.dt.float32)
            nc.vector.memset(bias_t, -thr_dev)
            # Interleave vq pieces with vt parts so the first matmuls gate on
            # a small DMA prefix and vt streams ahead of the wave-ascending
            # consumption order.  Inputs own the HWDGE queues (early outputs
            # go via SWDGE) so the stream is never starved.
            part_edges = [0, 512, 1024, 2048]
            while part_edges[-1] < n_cols:
                part_edges.append(min(n_cols, part_edges[-1] + 2048))
            vq_parts = 4
            vqw = rows // vq_parts
            for p in range(max(len(part_edges) - 1, vq_parts)):
                for c in range(2):
                    eng = nc.sync
                    if p < vq_parts:
                        eng.dma_start(
                            out=vq_sb[:, c, p * vqw:(p + 1) * vqw],
                            in_=vq_d[c, :, p * vqw:(p + 1) * vqw])
                    if p < len(part_edges) - 1:
                        lo, hi = part_edges[p], part_edges[p + 1]
                        eng.dma_start(
                            out=vt_sb[:, c, lo:hi],
                            in_=vt_d[c, :, lo:hi])

            i_mask = i_max = 0
            for k, ((s, jt0, n_jt), kind) in enumerate(zip(units, kinds)):
                fd = n_jt * J_TILE
                ts = slice(s * I_TILE, (s + 1) * I_TILE)
                ps = psum_pool.tile([128, nseg, SEG], mybir.dt.float32)
                for jj in range(n_jt):
                    js = slice((jt0 + jj) * J_TILE, (jt0 + jj + 1) * J_TILE)
                    nc.tensor.matmul(
                        ps[:, jj * (J_TILE // SEG):(jj + 1) * (J_TILE // SEG), :],
                        lhsT=vq_sb[:, :, ts], rhs=vt_sb[:, :, js],
                        start=True, stop=True,
                        perf_mode=mybir.MatmulPerfMode.DoubleRow,
                    )
                nsg = fd // SEG
                # Alternate output DMAs between GpSimd (SWDGE) and Sync
                # (HWDGE) so neither queue engine's descriptor issue rate
                # becomes the wall.
                dma_eng = nc.gpsimd if k % 2 == 0 else nc.sync
                if kind == "mask":
                    stage = stage_pool.tile([128, nseg, SEG], mybir.dt.uint8)
                    # Sign(sims - thr'): +1 above threshold; 0/255 otherwise
                    # (f32->u8 of -1 may wrap). Host treats ==1 as candidate.
                    nc.scalar.activation(
                        stage[:, :nsg, :], ps[:, :nsg, :],
                        mybir.ActivationFunctionType.Sign, bias=bias_t)
                    dma_eng.dma_start(
                        out=mask_d[i_mask, :, :fd],
                        in_=stage[:, :nsg, :])
                    i_mask += 1
                else:
                    mxt = mx_pool.tile([128, nseg], mybir.dt.float32)
                    nc.vector.tensor_reduce(
                        mxt[:, :nsg], ps[:, :nsg, :],
                        axis=mybir.AxisListType.X, op=mybir.AluOpType.max)
                    dma_eng.dma_start(out=mx_d[i_max, :, :nsg], in_=mxt[:, :nsg])
                    i_max += 1
    nc.finalize()
    return nc


def _device_candidate_edges(V32: np.ndarray, thr: float):
    """Run the SPMD kernel on 8 cores; return candidate pairs (ci, cj) with
    sims_fp8 >= thr - EPS, restricted to the computed upper-triangle blocks
    (a superset of every pair the reference's column mask admits).  Vector
    (reduce-max) units contribute whole 128-col segments per flagged row."""
    global LAST_EXEC_NS
    n = V32.shape[0]
    thr_dev = float(thr) - EPS

    key = (n, round(thr_dev, 9))
    if key not in _BUILD_CACHE:
        _BUILD_CACHE[key] = _build_program(n, thr_dev)
    nc = _BUILD_CACHE[key]

    vt8 = np.ascontiguousarray(
        V32.T.reshape(2, 128, n).astype(ml_dtypes.float8_e4m3))
    in_maps = []
    for c in range(N_CORES):
        cols = np.concatenate([
            np.arange(I_TILE * _itile_for_slot(c, s),
                      I_TILE * (_itile_for_slot(c, s) + 1))
            for s in range(SLOTS)])
        vq8 = np.ascontiguousarray(vt8[:, :, cols])
        in_maps.append({"vt": vt8, "vq": vq8})

    if TRACE:
        _ensure_ntff_hook()
    res = run_bass_kernel_spmd(
        nc, in_maps, core_ids=list(range(N_CORES)), trace=TRACE)
    if TRACE:
        LAST_EXEC_NS = res.exec_time_ns

    units = _unit_layout(n // J_TILE)
    kinds = _assign_engines(units)
    ci_all, cj_all = [], []
    for c in range(N_CORES):
        o_mask = res.results[c]["mask"]  # [n_mask, 128, gw]
        o_mx = res.results[c]["mx"]      # [n_max, 128, nseg]
        t_for_s = np.array([_itile_for_slot(c, s) for s in range(SLOTS)],
                           dtype=np.int64)
        i_mask = i_max = 0
        for (s, jt0, n_jt), kind in zip(units, kinds):
            fd = n_jt * J_TILE
            base_i = I_TILE * t_for_s[s]
            base_j = J_TILE * jt0
            if kind == "mask":
                o = o_mask[i_mask][:, :fd]
                bp, bq = np.nonzero(o == 1)
                if bp.size:
                    ci_all.append(base_i + bp)
                    cj_all.append(base_j + bq)
                i_mask += 1
            else:
                nsg = fd // SEG
                m = o_mx[i_max][:, :nsg]
                bp, bs = np.nonzero(m >= thr_dev)
                if bp.size:
                    # expand each flagged segment to its SEG columns
                    ci_all.append(np.repeat(base_i + bp, SEG))
                    cj_all.append(
                        (base_j + bs[:, None] * SEG
                         + np.arange(SEG)[None, :]).reshape(-1))
                i_max += 1
    if not ci_all:
        return (np.zeros(0, np.int64), np.zeros(0, np.int64))
    return np.concatenate(ci_all), np.concatenate(cj_all)


def _exact_edges(V32, ci, cj, thr, B):
    """From candidate pairs, produce exact reference edges:
    fp32 sims >= thr and j >= (i//B)*B + 1.  Returns (ci, cj)."""
    keep = cj >= (ci // B) * B + 1
    ci, cj = ci[keep], cj[keep]
    if ci.size:
        sims = np.empty(ci.size, np.float32)
        CH = 1 << 19
        for lo in range(0, ci.size, CH):
            hi = min(lo + CH, ci.size)
            sims[lo:hi] = np.einsum(
                "ij,ij->i", V32[ci[lo:hi]], V32[cj[lo:hi]])
        keep = sims >= np.float32(thr)
        ci, cj = ci[keep], cj[keep]
    return ci, cj


def _merge_replay(g, ci, cj, B):
    """Faithful replay of the reference's sequential merge.

    Per batch: the matched sets are frozen at batch start (with the
    g_i0 != g_j filter evaluated on batch-start group ids), then rows are
    processed sequentially; each row i merges every row whose CURRENT group
    id appears among the CURRENT group ids of its matched j's into i's
    CURRENT group."""
    n = g.shape[0]
    if ci.size == 0:
        return g
    order = np.argsort(ci, kind="stable")
    ci, cj = ci[order], cj[order]
    row_ids, row_starts = np.unique(ci, return_index=True)
    row_ends = np.append(row_starts[1:], ci.size)
    row_j = {int(i): cj[s:e] for i, s, e in zip(row_ids, row_starts, row_ends)}

    flag = np.zeros(max(n, int(g.max()) + 1), dtype=bool)
    for b in np.unique(row_ids // B):
        bs = int(b) * B
        g0 = g.copy()
        frozen = []
        for i in range(bs, bs + B):
            J = row_j.get(i)
            if J is None:
                continue
            J = J[g0[J] != g0[i]]
            if J.size:
                frozen.append((i, J))
        for i, J in frozen:
            mg = np.unique(g[J])
            flag[mg] = True
            sel = flag[g]
            g[sel] = g[i]
            flag[mg] = False
    return g


def kernel(V, group_ids, cos_threshold, batch_size):
    V32 = np.ascontiguousarray(np.asarray(V, dtype=np.float32))
    g = np.asarray(group_ids, dtype=np.int32).copy()
    thr = float(np.asarray(cos_threshold).reshape(-1)[0])
    B = int(np.asarray(batch_size))

    ci, cj = _device_candidate_edges(V32, thr)
    ci, cj = _exact_edges(V32, ci, cj, thr, B)
    g = _merge_replay(g, ci, cj, B)
    return g.astype(np.int32)


# revision 10
# speedup vs baseline: 1.2258x; 1.0982x over previous
"""Embedding-similarity group merge on 8 Trainium2 NeuronCores.

Strategy (v2: fp8 DoubleRow + split psum scan)
----------------------------------------------
The heavy part of the reference (Embeddings._fast_predict) is the blocked
cosine-similarity score computation V @ V.T (16384 x 16384 x 256).  The
transitive group-merge that follows is sequential and path-dependent but only
touches the ~10k above-threshold pairs, so it is replayed exactly on host.

Device work per core (SPMD, identical program; per-core behaviour comes only
from the vq input = that core's 16 interleaved query i-tiles):

* Matmul in fp8e4 with perf_mode=DoubleRow: the PE array virtualizes to
  128x256, so the full D=256 contraction happens in ONE matmul per
  [128 x 512] output tile at 2 fp8 elements/cycle -- 2x the bf16 rate.
  With both operands rounded to fp8e4, |sims_fp8 - sims_fp32| < EPS = 0.013
  (validated empirically; error std ~1e-3, max ~7e-3), so thresholding at
  thr - EPS yields a guaranteed superset of the true fp32 matches.
* The psum scan (the roofline wall: only Vector and Scalar can read PSUM,
  both at 1 element/cycle/lane) is split per 4-bank psum supertile between:
    - Scalar: Sign(sims - thr') -> u8 mask [128, 2048] -> DMA (exact cols)
    - Vector: segmented reduce_max -> [128, 16] fp32 (128-col segments),
      host re-expands flagged segments (tiny DMA instead of 256KB)
  Units are assigned greedily to balance the two engines' cycle counts.

Host: expands/gathers candidate pairs, recomputes their sims exactly in
fp32, applies the reference's column mask, and replays the reference's
sequential batch/row merge to produce bit-identical group ids.
"""

import sys

if "/opt/trn_rl_repo" not in sys.path:
    sys.path.insert(0, "/opt/trn_rl_repo")

import numpy as np
import ml_dtypes

import concourse.bass as bass
import concourse.tile as tile
from concourse import bacc, mybir
from concourse.bass_utils import run_bass_kernel_spmd

N_CORES = 8
D = 256                     # embedding dim (2 chunks of 128 on partitions)
EPS = 0.030                 # fp8e4 guard band (measured max err 0.023 @67M pairs)
I_TILE = 128                # psum partition tile (query rows per matmul)
J_TILE = 512                # matmul free-dim tile (one psum bank, fp32)
WAVE = 2                    # j-tiles per psum unit (2 banks; 4 units in flight)
SEG = 128                   # reduce-max segment width (cols)
SLOTS = 16                  # i-tiles per core

_BUILD_CACHE: dict = {}
LAST_EXEC_NS = None         # set when kernel() runs with TRACE=True
TRACE = False


def _itile_for_slot(c: int, s: int) -> int:
    """Global i-tile handled by core c in slot s (uniform-jstart interleave)."""
    k, r = divmod(s, 2)
    return 16 * k + (c if r == 0 else 15 - c)


def _jstart_for_slot(s: int) -> int:
    k, r = divmod(s, 2)
    return 4 * k + 2 * r


def _unit_layout(n_jtiles: int):
    """Program-order scan units, wave-ascending: (slot, jt0, n_jt)."""
    units = []
    n_waves = n_jtiles // WAVE
    for w in range(n_waves):
        for s in range(SLOTS):
            j0 = _jstart_for_slot(s)
            w0 = j0 // WAVE
            if w < w0:
                continue
            if w == w0:
                units.append((s, j0, (w0 + 1) * WAVE - j0))
            else:
                units.append((s, w * WAVE, WAVE))
    return units


def _assign_engines(units):
    """Greedy balance of scan cost: vector reduce vs scalar mask."""
    kinds = []
    v_acc = s_acc = 0.0
    for (_s, _j0, n_jt) in units:
        fd = n_jt * J_TILE
        v_cost = (120.0 + fd) / 0.96
        s_cost = (352.0 + fd) / 1.2
        if v_acc + v_cost <= s_acc + s_cost:
            kinds.append("max")
            v_acc += v_cost
        else:
            kinds.append("mask")
            s_acc += s_cost
    return kinds


def _ensure_ntff_hook():
    """Register the axon NTFF-profile hook (test/trace path only).

    The agent image's ``antenv`` lacks ``axon_hooks``, so ``trn_boot.boot``
    silently skips hook registration and ``bass_utils`` would crash on the
    import. Seed ``sys.modules['antenv.axon_hooks']`` with a stub wired to
    the ctypes hook so ``trace=True`` yields real NTFF profiles."""
    import types
    if "antenv.axon_hooks" in sys.modules:
        return
    try:
        from trn_agent_boot.trn_boot import _ntff_profile_via_ctypes
        hook = _ntff_profile_via_ctypes("/opt/axon/libaxon_pjrt.so")
    except Exception:
        hook = None
    mod = types.ModuleType("antenv.axon_hooks")
    mod._HOOK = hook
    mod.get_axon_ntff_profile_hook = lambda: mod._HOOK
    mod.set_axon_ntff_profile_hook = lambda h: setattr(mod, "_HOOK", h)
    sys.modules["antenv.axon_hooks"] = mod


def _build_program(n_cols: int, thr_dev: float) -> bass.Bass:
    """One SPMD program, identical across cores.

    Inputs (per core):
      vt [2, 128, n_cols] fp8e4 -- V.T split into two 128-row d-chunks
      vq [2, 128, 2048] fp8e4   -- this core's 16 i-tiles of query columns
    Outputs:
      mask [n_mask, 128, WAVE*J_TILE] u8    -- scalar-engine candidate masks
      mx   [n_max, 128, WAVE*J_TILE//SEG] f32 -- vector-engine segment maxes
    """
    n_jtiles = n_cols // J_TILE
    units = _unit_layout(n_jtiles)
    kinds = _assign_engines(units)
    rows = SLOTS * I_TILE
    gw = WAVE * J_TILE
    nseg = gw // SEG
    n_mask = sum(1 for k in kinds if k == "mask")
    n_max = len(kinds) - n_mask

    nb_mask = (n_mask + 3) // 4
    nb_max = (n_max + 3) // 4
    nc = bacc.Bacc(None, target_bir_lowering=False)
    vt_d = nc.declare_dram_parameter("vt", [2, 128, n_cols], mybir.dt.float8e4, isOutput=False)
    vq_d = nc.declare_dram_parameter("vq", [2, 128, rows], mybir.dt.float8e4, isOutput=False)
    mask_d = nc.declare_dram_parameter(
        "mask", [max(nb_mask, 1), I_TILE, 4, gw], mybir.dt.uint8, isOutput=True)
    mx_d = nc.declare_dram_parameter(
        "mx", [max(nb_max, 1), I_TILE, 4, nseg], mybir.dt.float32, isOutput=True)

    with tile.TileContext(nc) as tc:
        with (
            tc.tile_pool(name="vt", bufs=1) as vt_pool,
            tc.tile_pool(name="vq", bufs=1) as vq_pool,
            tc.tile_pool(name="psum", bufs=4, space="PSUM") as psum_pool,
            tc.tile_pool(name="stage", bufs=6) as stage_pool,
            tc.tile_pool(name="mxs", bufs=6) as mx_pool,
        ):
            vt_sb = vt_pool.tile([128, 2, n_cols], mybir.dt.float8e4)
            vq_sb = vq_pool.tile([128, 2, rows], mybir.dt.float8e4)
            bias_t = vq_pool.tile([128, 1], mybir.dt.float32)
            nc.vector.memset(bias_t, -thr_dev)
            # Interleave vq pieces with vt parts so the first matmuls gate on
            # a small DMA prefix and vt streams ahead of the wave-ascending
            # consumption order.  Inputs own the HWDGE queues (early outputs
            # go via SWDGE) so the stream is never starved.
            part_edges = [0, 512, 1024, 2048]
            while part_edges[-1] < n_cols:
                part_edges.append(min(n_cols, part_edges[-1] + 2048))
            vq_parts = 4
            vqw = rows // vq_parts
            for p in range(max(len(part_edges) - 1, vq_parts)):
                for c in range(2):
                    eng = nc.sync
                    if p < vq_parts:
                        eng.dma_start(
                            out=vq_sb[:, c, p * vqw:(p + 1) * vqw],
                            in_=vq_d[c, :, p * vqw:(p + 1) * vqw])
                    if p < len(part_edges) - 1:
                        lo, hi = part_edges[p], part_edges[p + 1]
                        eng.dma_start(
                            out=vt_sb[:, c, lo:hi],
                            in_=vt_d[c, :, lo:hi])

            i_mask = i_max = 0
            stage = mxt = None
            for k, ((s, jt0, n_jt), kind) in enumerate(zip(units, kinds)):
                fd = n_jt * J_TILE
                ts = slice(s * I_TILE, (s + 1) * I_TILE)
                ps = psum_pool.tile([128, nseg, SEG], mybir.dt.float32)
                for jj in range(n_jt):
                    js = slice((jt0 + jj) * J_TILE, (jt0 + jj + 1) * J_TILE)
                    nc.tensor.matmul(
                        ps[:, jj * (J_TILE // SEG):(jj + 1) * (J_TILE // SEG), :],
                        lhsT=vq_sb[:, :, ts], rhs=vt_sb[:, :, js],
                        start=True, stop=True,
                        perf_mode=mybir.MatmulPerfMode.DoubleRow,
                    )
                nsg = fd // SEG
                # Outputs are written into batch-of-4 staging tiles and
                # shipped with ONE DMA descriptor per 4 units: descriptor
                # issue costs ~0.7us of engine time each, so per-unit DMAs
                # would make the Sync/GpSimd engines a second wall (and the
                # extra engine activity pushes the chip into its P0 power
                # throttle, downclocking everything ~17%).
                if kind == "mask":
                    if stage is None:
                        stage = stage_pool.tile([128, 4, nseg, SEG], mybir.dt.uint8)
                    b = i_mask % 4
                    # Sign(sims - thr'): +1 above threshold; 0/255 otherwise
                    # (f32->u8 of -1 may wrap). Host treats ==1 as candidate.
                    nc.scalar.activation(
                        stage[:, b, :nsg, :], ps[:, :nsg, :],
                        mybir.ActivationFunctionType.Sign, bias=bias_t)
                    if b == 3 or i_mask == n_mask - 1:
                        nc.sync.dma_start(
                            out=mask_d[i_mask // 4], in_=stage)
                        stage = None
                    i_mask += 1
                else:
                    if mxt is None:
                        mxt = mx_pool.tile([128, 4, nseg], mybir.dt.float32)
                    b = i_max % 4
                    nc.vector.tensor_reduce(
                        mxt[:, b, :nsg], ps[:, :nsg, :],
                        axis=mybir.AxisListType.X, op=mybir.AluOpType.max)
                    if b == 3 or i_max == n_max - 1:
                        nc.sync.dma_start(out=mx_d[i_max // 4], in_=mxt)
                        mxt = None
                    i_max += 1
    nc.finalize()
    return nc


def _device_candidate_edges(V32: np.ndarray, thr: float):
    """Run the SPMD kernel on 8 cores; return candidate pairs (ci, cj) with
    sims_fp8 >= thr - EPS, restricted to the computed upper-triangle blocks
    (a superset of every pair the reference's column mask admits).  Vector
    (reduce-max) units contribute whole 128-col segments per flagged row."""
    global LAST_EXEC_NS
    n = V32.shape[0]
    thr_dev = float(thr) - EPS

    key = (n, round(thr_dev, 9))
    if key not in _BUILD_CACHE:
        _BUILD_CACHE[key] = _build_program(n, thr_dev)
    nc = _BUILD_CACHE[key]

    vt8 = np.ascontiguousarray(
        V32.T.reshape(2, 128, n).astype(ml_dtypes.float8_e4m3))
    in_maps = []
    for c in range(N_CORES):
        cols = np.concatenate([
            np.arange(I_TILE * _itile_for_slot(c, s),
                      I_TILE * (_itile_for_slot(c, s) + 1))
            for s in range(SLOTS)])
        vq8 = np.ascontiguousarray(vt8[:, :, cols])
        in_maps.append({"vt": vt8, "vq": vq8})

    if TRACE:
        _ensure_ntff_hook()
    res = run_bass_kernel_spmd(
        nc, in_maps, core_ids=list(range(N_CORES)), trace=TRACE)
    if TRACE:
        LAST_EXEC_NS = res.exec_time_ns

    units = _unit_layout(n // J_TILE)
    kinds = _assign_engines(units)
    ci_all, cj_all = [], []
    for c in range(N_CORES):
        o_mask = res.results[c]["mask"]  # [nb_mask, 128, 4, gw]
        o_mx = res.results[c]["mx"]      # [nb_max, 128, 4, nseg]
        t_for_s = np.array([_itile_for_slot(c, s) for s in range(SLOTS)],
                           dtype=np.int64)
        i_mask = i_max = 0
        for (s, jt0, n_jt), kind in zip(units, kinds):
            fd = n_jt * J_TILE
            base_i = I_TILE * t_for_s[s]
            base_j = J_TILE * jt0
            if kind == "mask":
                o = o_mask[i_mask // 4][:, i_mask % 4, :fd]
                bp, bq = np.nonzero(o == 1)
                if bp.size:
                    ci_all.append(base_i + bp)
                    cj_all.append(base_j + bq)
                i_mask += 1
            else:
                nsg = fd // SEG
                m = o_mx[i_max // 4][:, i_max % 4, :nsg]
                bp, bs = np.nonzero(m >= thr_dev)
                if bp.size:
                    # expand each flagged segment to its SEG columns
                    ci_all.append(np.repeat(base_i + bp, SEG))
                    cj_all.append(
                        (base_j + bs[:, None] * SEG
                         + np.arange(SEG)[None, :]).reshape(-1))
                i_max += 1
    if not ci_all:
        return (np.zeros(0, np.int64), np.zeros(0, np.int64))
    return np.concatenate(ci_all), np.concatenate(cj_all)


def _exact_edges(V32, ci, cj, thr, B):
    """From candidate pairs, produce exact reference edges:
    fp32 sims >= thr and j >= (i//B)*B + 1.  Returns (ci, cj)."""
    keep = cj >= (ci // B) * B + 1
    ci, cj = ci[keep], cj[keep]
    if ci.size:
        sims = np.empty(ci.size, np.float32)
        CH = 1 << 19
        for lo in range(0, ci.size, CH):
            hi = min(lo + CH, ci.size)
            sims[lo:hi] = np.einsum(
                "ij,ij->i", V32[ci[lo:hi]], V32[cj[lo:hi]])
        keep = sims >= np.float32(thr)
        ci, cj = ci[keep], cj[keep]
    return ci, cj


def _merge_replay(g, ci, cj, B):
    """Faithful replay of the reference's sequential merge.

    Per batch: the matched sets are frozen at batch start (with the
    g_i0 != g_j filter evaluated on batch-start group ids), then rows are
    processed sequentially; each row i merges every row whose CURRENT group
    id appears among the CURRENT group ids of its matched j's into i's
    CURRENT group."""
    n = g.shape[0]
    if ci.size == 0:
        return g
    order = np.argsort(ci, kind="stable")
    ci, cj = ci[order], cj[order]
    row_ids, row_starts = np.unique(ci, return_index=True)
    row_ends = np.append(row_starts[1:], ci.size)
    row_j = {int(i): cj[s:e] for i, s, e in zip(row_ids, row_starts, row_ends)}

    flag = np.zeros(max(n, int(g.max()) + 1), dtype=bool)
    for b in np.unique(row_ids // B):
        bs = int(b) * B
        g0 = g.copy()
        frozen = []
        for i in range(bs, bs + B):
            J = row_j.get(i)
            if J is None:
                continue
            J = J[g0[J] != g0[i]]
            if J.size:
                frozen.append((i, J))
        for i, J in frozen:
            mg = np.unique(g[J])
            flag[mg] = True
            sel = flag[g]
            g[sel] = g[i]
            flag[mg] = False
    return g


def kernel(V, group_ids, cos_threshold, batch_size):
    V32 = np.ascontiguousarray(np.asarray(V, dtype=np.float32))
    g = np.asarray(group_ids, dtype=np.int32).copy()
    thr = float(np.asarray(cos_threshold).reshape(-1)[0])
    B = int(np.asarray(batch_size))

    ci, cj = _device_candidate_edges(V32, thr)
    ci, cj = _exact_edges(V32, ci, cj, thr, B)
    g = _merge_replay(g, ci, cj, B)
    return g.astype(np.int32)


# revision 15
# speedup vs baseline: 1.4218x; 1.1599x over previous
"""Embedding-similarity group merge on 8 Trainium2 NeuronCores.

Strategy (v2: fp8 DoubleRow + split psum scan)
----------------------------------------------
The heavy part of the reference (Embeddings._fast_predict) is the blocked
cosine-similarity score computation V @ V.T (16384 x 16384 x 256).  The
transitive group-merge that follows is sequential and path-dependent but only
touches the ~10k above-threshold pairs, so it is replayed exactly on host.

Device work per core (SPMD, identical program; per-core behaviour comes only
from the vq input = that core's 16 interleaved query i-tiles):

* Matmul in fp8e4 with perf_mode=DoubleRow: the PE array virtualizes to
  128x256, so the full D=256 contraction happens in ONE matmul per
  [128 x 512] output tile at 2 fp8 elements/cycle -- 2x the bf16 rate.
  With both operands rounded to fp8e4, |sims_fp8 - sims_fp32| < EPS = 0.013
  (validated empirically; error std ~1e-3, max ~7e-3), so thresholding at
  thr - EPS yields a guaranteed superset of the true fp32 matches.
* The psum scan (the roofline wall: only Vector and Scalar can read PSUM,
  both at 1 element/cycle/lane) is split per 4-bank psum supertile between:
    - Scalar: Sign(sims - thr') -> u8 mask [128, 2048] -> DMA (exact cols)
    - Vector: segmented reduce_max -> [128, 16] fp32 (128-col segments),
      host re-expands flagged segments (tiny DMA instead of 256KB)
  Units are assigned greedily to balance the two engines' cycle counts.

Host: expands/gathers candidate pairs, recomputes their sims exactly in
fp32, applies the reference's column mask, and replays the reference's
sequential batch/row merge to produce bit-identical group ids.
"""

import sys

if "/opt/trn_rl_repo" not in sys.path:
    sys.path.insert(0, "/opt/trn_rl_repo")

import numpy as np
import ml_dtypes

import concourse.bass as bass
import concourse.tile as tile
from concourse import bacc, mybir
from concourse.bass_utils import run_bass_kernel_spmd

N_CORES = 8
D = 256                     # embedding dim (2 chunks of 128 on partitions)
EPS = 0.030                 # fp8e4 guard band (measured max err 0.023 @67M pairs)
I_TILE = 128                # psum partition tile (query rows per matmul)
J_TILE = 512                # matmul free-dim tile (one psum bank, fp32)
WAVE = 2                    # j-tiles per psum unit (2 banks; 4 units in flight)
SEG = 128                   # reduce-max segment width (cols)
SLOTS = 16                  # i-tiles per core

_BUILD_CACHE: dict = {}
LAST_EXEC_NS = None         # set when kernel() runs with TRACE=True
TRACE = False


def _itile_for_slot(c: int, s: int) -> int:
    """Global i-tile handled by core c in slot s (uniform-jstart interleave)."""
    k, r = divmod(s, 2)
    return 16 * k + (c if r == 0 else 15 - c)


def _jstart_for_slot(s: int) -> int:
    k, r = divmod(s, 2)
    return 4 * k + 2 * r


def _unit_layout(n_jtiles: int):
    """Program-order scan units, wave-ascending: (slot, jt0, n_jt)."""
    units = []
    n_waves = n_jtiles // WAVE
    for w in range(n_waves):
        for s in range(SLOTS):
            j0 = _jstart_for_slot(s)
            w0 = j0 // WAVE
            if w < w0:
                continue
            if w == w0:
                units.append((s, j0, (w0 + 1) * WAVE - j0))
            else:
                units.append((s, w * WAVE, WAVE))
    return units


def _assign_engines(units):
    """Greedy balance of scan cost: vector reduce vs scalar mask.

    Rates are the MEASURED per-op costs under the P0 power state the kernel
    runs in (all clocks at 5/6 nominal): DVE 0.8 GHz, ACT 1.0 GHz."""
    kinds = []
    v_acc = s_acc = 0.0
    for (_s, _j0, n_jt) in units:
        fd = n_jt * J_TILE
        v_cost = (147.0 + fd) / 0.80
        s_cost = (312.0 + fd) / 1.00
        if v_acc + v_cost <= s_acc + s_cost:
            kinds.append("max")
            v_acc += v_cost
        else:
            kinds.append("mask")
            s_acc += s_cost
    return kinds


def _ensure_ntff_hook():
    """Register the axon NTFF-profile hook (test/trace path only).

    The agent image's ``antenv`` lacks ``axon_hooks``, so ``trn_boot.boot``
    silently skips hook registration and ``bass_utils`` would crash on the
    import. Seed ``sys.modules['antenv.axon_hooks']`` with a stub wired to
    the ctypes hook so ``trace=True`` yields real NTFF profiles."""
    import types
    if "antenv.axon_hooks" in sys.modules:
        return
    try:
        from trn_agent_boot.trn_boot import _ntff_profile_via_ctypes
        hook = _ntff_profile_via_ctypes("/opt/axon/libaxon_pjrt.so")
    except Exception:
        hook = None
    mod = types.ModuleType("antenv.axon_hooks")
    mod._HOOK = hook
    mod.get_axon_ntff_profile_hook = lambda: mod._HOOK
    mod.set_axon_ntff_profile_hook = lambda h: setattr(mod, "_HOOK", h)
    sys.modules["antenv.axon_hooks"] = mod


def _build_program(n_cols: int, thr_dev: float) -> bass.Bass:
    """One SPMD program, identical across cores.

    Inputs (per core):
      vt [2, 128, n_cols] fp8e4 -- V.T split into two 128-row d-chunks
      vq [2, 128, 2048] fp8e4   -- this core's 16 i-tiles of query columns
    Outputs:
      mask [n_mask, 128, WAVE*J_TILE] u8    -- scalar-engine candidate masks
      mx   [n_max, 128, WAVE*J_TILE//SEG] f32 -- vector-engine segment maxes
    """
    n_jtiles = n_cols // J_TILE
    units = _unit_layout(n_jtiles)
    kinds = _assign_engines(units)
    rows = SLOTS * I_TILE
    gw = WAVE * J_TILE
    nseg = gw // SEG
    n_mask = sum(1 for k in kinds if k == "mask")
    n_max = len(kinds) - n_mask

    nb_mask = (n_mask + 3) // 4
    nb_max = (n_max + 3) // 4
    nc = bacc.Bacc(None, target_bir_lowering=False)
    vt_d = nc.declare_dram_parameter("vt", [2, 128, n_cols], mybir.dt.float8e4, isOutput=False)
    vq_d = nc.declare_dram_parameter("vq", [2, 128, rows], mybir.dt.float8e4, isOutput=False)
    mask_d = nc.declare_dram_parameter(
        "mask", [max(nb_mask, 1), I_TILE, 4, gw], mybir.dt.uint8, isOutput=True)
    mx_d = nc.declare_dram_parameter(
        "mx", [max(nb_max, 1), I_TILE, 4, nseg], mybir.dt.float32, isOutput=True)

    with tile.TileContext(nc) as tc:
        with (
            tc.tile_pool(name="vt", bufs=1) as vt_pool,
            tc.tile_pool(name="vq", bufs=1) as vq_pool,
            tc.tile_pool(name="psum", bufs=4, space="PSUM") as psum_pool,
            tc.tile_pool(name="stage", bufs=6) as stage_pool,
            tc.tile_pool(name="mxs", bufs=6) as mx_pool,
        ):
            vt_sb = vt_pool.tile([128, 2, n_cols], mybir.dt.float8e4)
            vq_sb = vq_pool.tile([128, 2, rows], mybir.dt.float8e4)
            bias_t = vq_pool.tile([128, 1], mybir.dt.float32)
            scratch = vq_pool.tile([128, 2, J_TILE], mybir.dt.float8e4)
            nc.vector.memset(bias_t, -thr_dev)
            nc.vector.memset(scratch, 0)
            # HAM warmup: the PE clock gate defaults to K=4/8 (half rate)
            # and needs ~3.4us of sustained matmul activity to open.  The
            # first real matmuls are gated on the input DMA and arrive too
            # sparsely to warm it, so the early compute would run 2x slow.
            # Issue ~12 dependency-free matmuls on zeroed scratch during the
            # DMA head; their psum tile is never read.
            warm_ps = psum_pool.tile([128, nseg, SEG], mybir.dt.float32,
                                     tag="ps")
            for _ in range(9):
                nc.tensor.matmul(
                    warm_ps[:, :J_TILE // SEG, :],
                    lhsT=scratch[:, :, :128], rhs=scratch,
                    start=True, stop=True,
                    perf_mode=mybir.MatmulPerfMode.DoubleRow,
                )
            # Interleave vq pieces with vt parts so the first matmuls gate on
            # a small DMA prefix and vt streams ahead of the wave-ascending
            # consumption order.  Inputs own the HWDGE queues (outputs are
            # batched 4 units per descriptor) so the stream is never starved.
            part_edges = [0, 512, 1024, 2048]
            while part_edges[-1] < n_cols:
                part_edges.append(min(n_cols, part_edges[-1] + 2048))
            vq_edges = [0, 128, 640, 1280, rows]
            for p in range(max(len(part_edges), len(vq_edges)) - 1):
                for c in range(2):
                    eng = nc.sync
                    if p < len(vq_edges) - 1:
                        qlo, qhi = vq_edges[p], vq_edges[p + 1]
                        eng.dma_start(
                            out=vq_sb[:, c, qlo:qhi],
                            in_=vq_d[c, :, qlo:qhi])
                    if p < len(part_edges) - 1:
                        lo, hi = part_edges[p], part_edges[p + 1]
                        eng.dma_start(
                            out=vt_sb[:, c, lo:hi],
                            in_=vt_d[c, :, lo:hi])

            i_mask = i_max = 0
            stage = mxt = None
            for k, ((s, jt0, n_jt), kind) in enumerate(zip(units, kinds)):
                fd = n_jt * J_TILE
                ts = slice(s * I_TILE, (s + 1) * I_TILE)
                ps = psum_pool.tile([128, nseg, SEG], mybir.dt.float32,
                                    tag="ps")
                for jj in range(n_jt):
                    js = slice((jt0 + jj) * J_TILE, (jt0 + jj + 1) * J_TILE)
                    nc.tensor.matmul(
                        ps[:, jj * (J_TILE // SEG):(jj + 1) * (J_TILE // SEG), :],
                        lhsT=vq_sb[:, :, ts], rhs=vt_sb[:, :, js],
                        start=True, stop=True,
                        perf_mode=mybir.MatmulPerfMode.DoubleRow,
                    )
                nsg = fd // SEG
                # Outputs are written into batch-of-4 staging tiles and
                # shipped with ONE DMA descriptor per 4 units: descriptor
                # issue costs ~0.7us of engine time each, so per-unit DMAs
                # would make the Sync/GpSimd engines a second wall (and the
                # extra engine activity pushes the chip into its P0 power
                # throttle, downclocking everything ~17%).
                if kind == "mask":
                    if stage is None:
                        stage = stage_pool.tile([128, 4, nseg, SEG], mybir.dt.uint8)
                    b = i_mask % 4
                    # Sign(sims - thr'): +1 above threshold; 0/255 otherwise
                    # (f32->u8 of -1 may wrap). Host treats ==1 as candidate.
                    nc.scalar.activation(
                        stage[:, b, :nsg, :], ps[:, :nsg, :],
                        mybir.ActivationFunctionType.Sign, bias=bias_t)
                    if b == 3 or i_mask == n_mask - 1:
                        nc.sync.dma_start(
                            out=mask_d[i_mask // 4], in_=stage)
                        stage = None
                    i_mask += 1
                else:
                    if mxt is None:
                        mxt = mx_pool.tile([128, 4, nseg], mybir.dt.float32)
                    b = i_max % 4
                    nc.vector.tensor_reduce(
                        mxt[:, b, :nsg], ps[:, :nsg, :],
                        axis=mybir.AxisListType.X, op=mybir.AluOpType.max)
                    if b == 3 or i_max == n_max - 1:
                        nc.sync.dma_start(out=mx_d[i_max // 4], in_=mxt)
                        mxt = None
                    i_max += 1
    nc.finalize()
    return nc


def _device_candidate_edges(V32: np.ndarray, thr: float):
    """Run the SPMD kernel on 8 cores; return candidate pairs (ci, cj) with
    sims_fp8 >= thr - EPS, restricted to the computed upper-triangle blocks
    (a superset of every pair the reference's column mask admits).  Vector
    (reduce-max) units contribute whole 128-col segments per flagged row."""
    global LAST_EXEC_NS
    n = V32.shape[0]
    thr_dev = float(thr) - EPS

    key = (n, round(thr_dev, 9))
    if key not in _BUILD_CACHE:
        _BUILD_CACHE[key] = _build_program(n, thr_dev)
    nc = _BUILD_CACHE[key]

    vt8 = np.ascontiguousarray(
        V32.T.reshape(2, 128, n).astype(ml_dtypes.float8_e4m3))
    in_maps = []
    for c in range(N_CORES):
        cols = np.concatenate([
            np.arange(I_TILE * _itile_for_slot(c, s),
                      I_TILE * (_itile_for_slot(c, s) + 1))
            for s in range(SLOTS)])
        vq8 = np.ascontiguousarray(vt8[:, :, cols])
        in_maps.append({"vt": vt8, "vq": vq8})

    if TRACE:
        _ensure_ntff_hook()
    res = run_bass_kernel_spmd(
        nc, in_maps, core_ids=list(range(N_CORES)), trace=TRACE)
    if TRACE:
        LAST_EXEC_NS = res.exec_time_ns

    units = _unit_layout(n // J_TILE)
    kinds = _assign_engines(units)
    ci_all, cj_all = [], []
    for c in range(N_CORES):
        o_mask = res.results[c]["mask"]  # [nb_mask, 128, 4, gw]
        o_mx = res.results[c]["mx"]      # [nb_max, 128, 4, nseg]
        t_for_s = np.array([_itile_for_slot(c, s) for s in range(SLOTS)],
                           dtype=np.int64)
        i_mask = i_max = 0
        for (s, jt0, n_jt), kind in zip(units, kinds):
            fd = n_jt * J_TILE
            base_i = I_TILE * t_for_s[s]
            base_j = J_TILE * jt0
            if kind == "mask":
                o = o_mask[i_mask // 4][:, i_mask % 4, :fd]
                bp, bq = np.nonzero(o == 1)
                if bp.size:
                    ci_all.append(base_i + bp)
                    cj_all.append(base_j + bq)
                i_mask += 1
            else:
                nsg = fd // SEG
                m = o_mx[i_max // 4][:, i_max % 4, :nsg]
                bp, bs = np.nonzero(m >= thr_dev)
                if bp.size:
                    # expand each flagged segment to its SEG columns
                    ci_all.append(np.repeat(base_i + bp, SEG))
                    cj_all.append(
                        (base_j + bs[:, None] * SEG
                         + np.arange(SEG)[None, :]).reshape(-1))
                i_max += 1
    if not ci_all:
        return (np.zeros(0, np.int64), np.zeros(0, np.int64))
    return np.concatenate(ci_all), np.concatenate(cj_all)


def _exact_edges(V32, ci, cj, thr, B):
    """From candidate pairs, produce exact reference edges:
    fp32 sims >= thr and j >= (i//B)*B + 1.  Returns (ci, cj)."""
    keep = cj >= (ci // B) * B + 1
    ci, cj = ci[keep], cj[keep]
    if ci.size:
        sims = np.empty(ci.size, np.float32)
        CH = 1 << 19
        for lo in range(0, ci.size, CH):
            hi = min(lo + CH, ci.size)
            sims[lo:hi] = np.einsum(
                "ij,ij->i", V32[ci[lo:hi]], V32[cj[lo:hi]])
        keep = sims >= np.float32(thr)
        ci, cj = ci[keep], cj[keep]
    return ci, cj


def _merge_replay(g, ci, cj, B):
    """Faithful replay of the reference's sequential merge.

    Per batch: the matched sets are frozen at batch start (with the
    g_i0 != g_j filter evaluated on batch-start group ids), then rows are
    processed sequentially; each row i merges every row whose CURRENT group
    id appears among the CURRENT group ids of its matched j's into i's
    CURRENT group."""
    n = g.shape[0]
    if ci.size == 0:
        return g
    order = np.argsort(ci, kind="stable")
    ci, cj = ci[order], cj[order]
    row_ids, row_starts = np.unique(ci, return_index=True)
    row_ends = np.append(row_starts[1:], ci.size)
    row_j = {int(i): cj[s:e] for i, s, e in zip(row_ids, row_starts, row_ends)}

    flag = np.zeros(max(n, int(g.max()) + 1), dtype=bool)
    for b in np.unique(row_ids // B):
        bs = int(b) * B
        g0 = g.copy()
        frozen = []
        for i in range(bs, bs + B):
            J = row_j.get(i)
            if J is None:
                continue
            J = J[g0[J] != g0[i]]
            if J.size:
                frozen.append((i, J))
        for i, J in frozen:
            mg = np.unique(g[J])
            flag[mg] = True
            sel = flag[g]
            g[sel] = g[i]
            flag[mg] = False
    return g


def kernel(V, group_ids, cos_threshold, batch_size):
    V32 = np.ascontiguousarray(np.asarray(V, dtype=np.float32))
    g = np.asarray(group_ids, dtype=np.int32).copy()
    thr = float(np.asarray(cos_threshold).reshape(-1)[0])
    B = int(np.asarray(batch_size))

    ci, cj = _device_candidate_edges(V32, thr)
    ci, cj = _exact_edges(V32, ci, cj, thr, B)
    g = _merge_replay(g, ci, cj, B)
    return g.astype(np.int32)


# revision 18
# speedup vs baseline: 1.4343x; 1.0088x over previous
"""Embedding-similarity group merge on 8 Trainium2 NeuronCores.

Strategy (v2: fp8 DoubleRow + split psum scan)
----------------------------------------------
The heavy part of the reference (Embeddings._fast_predict) is the blocked
cosine-similarity score computation V @ V.T (16384 x 16384 x 256).  The
transitive group-merge that follows is sequential and path-dependent but only
touches the ~10k above-threshold pairs, so it is replayed exactly on host.

Device work per core (SPMD, identical program; per-core behaviour comes only
from the vq input = that core's 16 interleaved query i-tiles):

* Matmul in fp8e4 with perf_mode=DoubleRow: the PE array virtualizes to
  128x256, so the full D=256 contraction happens in ONE matmul per
  [128 x 512] output tile at 2 fp8 elements/cycle -- 2x the bf16 rate.
  With both operands rounded to fp8e4, |sims_fp8 - sims_fp32| < EPS = 0.013
  (validated empirically; error std ~1e-3, max ~7e-3), so thresholding at
  thr - EPS yields a guaranteed superset of the true fp32 matches.
* The psum scan (the roofline wall: only Vector and Scalar can read PSUM,
  both at 1 element/cycle/lane) is split per 4-bank psum supertile between:
    - Scalar: Sign(sims - thr') -> u8 mask [128, 2048] -> DMA (exact cols)
    - Vector: segmented reduce_max -> [128, 16] fp32 (128-col segments),
      host re-expands flagged segments (tiny DMA instead of 256KB)
  Units are assigned greedily to balance the two engines' cycle counts.

Host: expands/gathers candidate pairs, recomputes their sims exactly in
fp32, applies the reference's column mask, and replays the reference's
sequential batch/row merge to produce bit-identical group ids.
"""

import sys

if "/opt/trn_rl_repo" not in sys.path:
    sys.path.insert(0, "/opt/trn_rl_repo")

import numpy as np
import ml_dtypes

import concourse.bass as bass
import concourse.tile as tile
from concourse import bacc, mybir
from concourse.bass_utils import run_bass_kernel_spmd

N_CORES = 8
D = 256                     # embedding dim (2 chunks of 128 on partitions)
EPS = 0.030                 # fp8e4 guard band (measured max err 0.023 @67M pairs)
I_TILE = 128                # psum partition tile (query rows per matmul)
J_TILE = 512                # matmul free-dim tile (one psum bank, fp32)
WAVE = 2                    # j-tiles per psum unit (2 banks; 4 units in flight)
SEG = 128                   # reduce-max segment width (cols)
SLOTS = 16                  # i-tiles per core

_BUILD_CACHE: dict = {}
LAST_EXEC_NS = None         # set when kernel() runs with TRACE=True
TRACE = False


def _itile_for_slot(c: int, s: int) -> int:
    """Global i-tile handled by core c in slot s (uniform-jstart interleave)."""
    k, r = divmod(s, 2)
    return 16 * k + (c if r == 0 else 15 - c)


def _jstart_for_slot(s: int) -> int:
    k, r = divmod(s, 2)
    return 4 * k + 2 * r


def _unit_layout(n_jtiles: int):
    """Program-order scan units, wave-ascending: (slot, jt0, n_jt).

    Waves are emitted in PAIRS with the slot loop outside, so consecutive
    units within a wave-pair share the same lhsT (query i-tile) and walrus
    can reuse the loaded PE weights: one LDWEIGHTS per 4 matmuls instead
    of per 2."""
    units = []
    n_waves = n_jtiles // WAVE
    for w0 in range(0, n_waves, 2):
        for s in range(SLOTS):
            j0 = _jstart_for_slot(s)
            for w in (w0, w0 + 1):
                if w >= n_waves or w * WAVE < j0:
                    continue
                units.append((s, w * WAVE, WAVE))
    return units


def _assign_engines(units):
    """Greedy balance of scan cost: vector reduce vs scalar mask.

    Rates are the MEASURED per-op costs under the P0 power state the kernel
    runs in (all clocks at 5/6 nominal): DVE 0.8 GHz, ACT 1.0 GHz."""
    kinds = []
    v_acc = s_acc = 0.0
    for (_s, _j0, n_jt) in units:
        fd = n_jt * J_TILE
        v_cost = (147.0 + fd) / 0.80
        s_cost = (312.0 + fd) / 1.00
        if v_acc + v_cost <= s_acc + s_cost:
            kinds.append("max")
            v_acc += v_cost
        else:
            kinds.append("mask")
            s_acc += s_cost
    return kinds


def _ensure_ntff_hook():
    """Register the axon NTFF-profile hook (test/trace path only).

    The agent image's ``antenv`` lacks ``axon_hooks``, so ``trn_boot.boot``
    silently skips hook registration and ``bass_utils`` would crash on the
    import. Seed ``sys.modules['antenv.axon_hooks']`` with a stub wired to
    the ctypes hook so ``trace=True`` yields real NTFF profiles."""
    import types
    if "antenv.axon_hooks" in sys.modules:
        return
    try:
        from trn_agent_boot.trn_boot import _ntff_profile_via_ctypes
        hook = _ntff_profile_via_ctypes("/opt/axon/libaxon_pjrt.so")
    except Exception:
        hook = None
    mod = types.ModuleType("antenv.axon_hooks")
    mod._HOOK = hook
    mod.get_axon_ntff_profile_hook = lambda: mod._HOOK
    mod.set_axon_ntff_profile_hook = lambda h: setattr(mod, "_HOOK", h)
    sys.modules["antenv.axon_hooks"] = mod


def _build_program(n_cols: int, thr_dev: float) -> bass.Bass:
    """One SPMD program, identical across cores.

    Inputs (per core):
      vt [2, 128, n_cols] fp8e4 -- V.T split into two 128-row d-chunks
      vq [2, 128, 2048] fp8e4   -- this core's 16 i-tiles of query columns
    Outputs:
      mask [n_mask, 128, WAVE*J_TILE] u8    -- scalar-engine candidate masks
      mx   [n_max, 128, WAVE*J_TILE//SEG] f32 -- vector-engine segment maxes
    """
    n_jtiles = n_cols // J_TILE
    units = _unit_layout(n_jtiles)
    kinds = _assign_engines(units)
    rows = SLOTS * I_TILE
    gw = WAVE * J_TILE
    nseg = gw // SEG
    n_mask = sum(1 for k in kinds if k == "mask")
    n_max = len(kinds) - n_mask

    nb_mask = (n_mask + 3) // 4
    nb_max = (n_max + 3) // 4
    nc = bacc.Bacc(None, target_bir_lowering=False)
    vt_d = nc.declare_dram_parameter("vt", [2, 128, n_cols], mybir.dt.float8e4, isOutput=False)
    vq_d = nc.declare_dram_parameter("vq", [2, 128, rows], mybir.dt.float8e4, isOutput=False)
    mask_d = nc.declare_dram_parameter(
        "mask", [max(nb_mask, 1), I_TILE, 4, gw], mybir.dt.uint8, isOutput=True)
    mx_d = nc.declare_dram_parameter(
        "mx", [max(nb_max, 1), I_TILE, 4, nseg], mybir.dt.float32, isOutput=True)

    with tile.TileContext(nc) as tc:
        with (
            tc.tile_pool(name="vt", bufs=1) as vt_pool,
            tc.tile_pool(name="vq", bufs=1) as vq_pool,
            tc.tile_pool(name="psum", bufs=4, space="PSUM") as psum_pool,
            tc.tile_pool(name="stage", bufs=6) as stage_pool,
            tc.tile_pool(name="mxs", bufs=6) as mx_pool,
        ):
            vt_sb = vt_pool.tile([128, 2, n_cols], mybir.dt.float8e4)
            vq_sb = vq_pool.tile([128, 2, rows], mybir.dt.float8e4)
            bias_t = vq_pool.tile([128, 1], mybir.dt.float32)
            scratch = vq_pool.tile([128, 2, J_TILE], mybir.dt.float8e4)
            nc.gpsimd.memset(scratch, 0)
            nc.vector.memset(bias_t, -thr_dev)
            # HAM warmup: the PE clock gate defaults to K=4/8 (half rate)
            # and needs ~3.4us of sustained matmul activity to open.  The
            # first real matmuls are gated on the input DMA and arrive too
            # sparsely to warm it, so the early compute would run 2x slow.
            # Issue ~12 dependency-free matmuls on zeroed scratch during the
            # DMA head; their psum tile is never read.
            warm_ps = psum_pool.tile([128, nseg, SEG], mybir.dt.float32,
                                     tag="ps")
            for _ in range(6):
                nc.tensor.matmul(
                    warm_ps[:, :J_TILE // SEG, :],
                    lhsT=scratch[:, :, :128], rhs=scratch,
                    start=True, stop=True,
                    perf_mode=mybir.MatmulPerfMode.DoubleRow,
                )
            # Interleave vq pieces with vt parts so the first matmuls gate on
            # a small DMA prefix and vt streams ahead of the wave-ascending
            # consumption order.  Inputs own the HWDGE queues (outputs are
            # batched 4 units per descriptor) so the stream is never starved.
            part_edges = [0, 512, 1024, 2048]
            while part_edges[-1] < n_cols:
                part_edges.append(min(n_cols, part_edges[-1] + 2048))
            vq_edges = [0, 128, 640, 1280, rows]
            for p in range(max(len(part_edges), len(vq_edges)) - 1):
                for c in range(2):
                    eng = nc.sync
                    if p < len(vq_edges) - 1:
                        qlo, qhi = vq_edges[p], vq_edges[p + 1]
                        eng.dma_start(
                            out=vq_sb[:, c, qlo:qhi],
                            in_=vq_d[c, :, qlo:qhi])
                    if p < len(part_edges) - 1:
                        lo, hi = part_edges[p], part_edges[p + 1]
                        eng.dma_start(
                            out=vt_sb[:, c, lo:hi],
                            in_=vt_d[c, :, lo:hi])

            i_mask = i_max = 0
            stage = mxt = None
            for k, ((s, jt0, n_jt), kind) in enumerate(zip(units, kinds)):
                fd = n_jt * J_TILE
                ts = slice(s * I_TILE, (s + 1) * I_TILE)
                ps = psum_pool.tile([128, nseg, SEG], mybir.dt.float32,
                                    tag="ps")
                for jj in range(n_jt):
                    js = slice((jt0 + jj) * J_TILE, (jt0 + jj + 1) * J_TILE)
                    nc.tensor.matmul(
                        ps[:, jj * (J_TILE // SEG):(jj + 1) * (J_TILE // SEG), :],
                        lhsT=vq_sb[:, :, ts], rhs=vt_sb[:, :, js],
                        start=True, stop=True,
                        perf_mode=mybir.MatmulPerfMode.DoubleRow,
                    )
                nsg = fd // SEG
                # Outputs are written into batch-of-4 staging tiles and
                # shipped with ONE DMA descriptor per 4 units: descriptor
                # issue costs ~0.7us of engine time each, so per-unit DMAs
                # would make the Sync/GpSimd engines a second wall (and the
                # extra engine activity pushes the chip into its P0 power
                # throttle, downclocking everything ~17%).
                if kind == "mask":
                    if stage is None:
                        stage = stage_pool.tile([128, 4, nseg, SEG], mybir.dt.uint8)
                    b = i_mask % 4
                    # Sign(sims - thr'): +1 above threshold; 0/255 otherwise
                    # (f32->u8 of -1 may wrap). Host treats ==1 as candidate.
                    nc.scalar.activation(
                        stage[:, b, :nsg, :], ps[:, :nsg, :],
                        mybir.ActivationFunctionType.Sign, bias=bias_t)
                    if b == 3 or i_mask == n_mask - 1:
                        nc.sync.dma_start(
                            out=mask_d[i_mask // 4], in_=stage)
                        stage = None
                    i_mask += 1
                else:
                    if mxt is None:
                        mxt = mx_pool.tile([128, 4, nseg], mybir.dt.float32)
                    b = i_max % 4
                    nc.vector.tensor_reduce(
                        mxt[:, b, :nsg], ps[:, :nsg, :],
                        axis=mybir.AxisListType.X, op=mybir.AluOpType.max)
                    if b == 3 or i_max == n_max - 1:
                        nc.sync.dma_start(out=mx_d[i_max // 4], in_=mxt)
                        mxt = None
                    i_max += 1
    nc.finalize()
    return nc


def _device_candidate_edges(V32: np.ndarray, thr: float):
    """Run the SPMD kernel on 8 cores; return candidate pairs (ci, cj) with
    sims_fp8 >= thr - EPS, restricted to the computed upper-triangle blocks
    (a superset of every pair the reference's column mask admits).  Vector
    (reduce-max) units contribute whole 128-col segments per flagged row."""
    global LAST_EXEC_NS
    n = V32.shape[0]
    thr_dev = float(thr) - EPS

    key = (n, round(thr_dev, 9))
    if key not in _BUILD_CACHE:
        _BUILD_CACHE[key] = _build_program(n, thr_dev)
    nc = _BUILD_CACHE[key]

    vt8 = np.ascontiguousarray(
        V32.T.reshape(2, 128, n).astype(ml_dtypes.float8_e4m3))
    in_maps = []
    for c in range(N_CORES):
        cols = np.concatenate([
            np.arange(I_TILE * _itile_for_slot(c, s),
                      I_TILE * (_itile_for_slot(c, s) + 1))
            for s in range(SLOTS)])
        vq8 = np.ascontiguousarray(vt8[:, :, cols])
        in_maps.append({"vt": vt8, "vq": vq8})

    if TRACE:
        _ensure_ntff_hook()
    res = run_bass_kernel_spmd(
        nc, in_maps, core_ids=list(range(N_CORES)), trace=TRACE)
    if TRACE:
        LAST_EXEC_NS = res.exec_time_ns

    units = _unit_layout(n // J_TILE)
    kinds = _assign_engines(units)
    ci_all, cj_all = [], []
    for c in range(N_CORES):
        o_mask = res.results[c]["mask"]  # [nb_mask, 128, 4, gw]
        o_mx = res.results[c]["mx"]      # [nb_max, 128, 4, nseg]
        t_for_s = np.array([_itile_for_slot(c, s) for s in range(SLOTS)],
                           dtype=np.int64)
        i_mask = i_max = 0
        for (s, jt0, n_jt), kind in zip(units, kinds):
            fd = n_jt * J_TILE
            base_i = I_TILE * t_for_s[s]
            base_j = J_TILE * jt0
            if kind == "mask":
                o = o_mask[i_mask // 4][:, i_mask % 4, :fd]
                bp, bq = np.nonzero(o == 1)
                if bp.size:
                    ci_all.append(base_i + bp)
                    cj_all.append(base_j + bq)
                i_mask += 1
            else:
                nsg = fd // SEG
                m = o_mx[i_max // 4][:, i_max % 4, :nsg]
                bp, bs = np.nonzero(m >= thr_dev)
                if bp.size:
                    # expand each flagged segment to its SEG columns
                    ci_all.append(np.repeat(base_i + bp, SEG))
                    cj_all.append(
                        (base_j + bs[:, None] * SEG
                         + np.arange(SEG)[None, :]).reshape(-1))
                i_max += 1
    if not ci_all:
        return (np.zeros(0, np.int64), np.zeros(0, np.int64))
    return np.concatenate(ci_all), np.concatenate(cj_all)


def _exact_edges(V32, ci, cj, thr, B):
    """From candidate pairs, produce exact reference edges:
    fp32 sims >= thr and j >= (i//B)*B + 1.  Returns (ci, cj)."""
    keep = cj >= (ci // B) * B + 1
    ci, cj = ci[keep], cj[keep]
    if ci.size:
        sims = np.empty(ci.size, np.float32)
        CH = 1 << 19
        for lo in range(0, ci.size, CH):
            hi = min(lo + CH, ci.size)
            sims[lo:hi] = np.einsum(
                "ij,ij->i", V32[ci[lo:hi]], V32[cj[lo:hi]])
        keep = sims >= np.float32(thr)
        ci, cj = ci[keep], cj[keep]
    return ci, cj


def _merge_replay(g, ci, cj, B):
    """Faithful replay of the reference's sequential merge.

    Per batch: the matched sets are frozen at batch start (with the
    g_i0 != g_j filter evaluated on batch-start group ids), then rows are
    processed sequentially; each row i merges every row whose CURRENT group
    id appears among the CURRENT group ids of its matched j's into i's
    CURRENT group."""
    n = g.shape[0]
    if ci.size == 0:
        return g
    order = np.argsort(ci, kind="stable")
    ci, cj = ci[order], cj[order]
    row_ids, row_starts = np.unique(ci, return_index=True)
    row_ends = np.append(row_starts[1:], ci.size)
    row_j = {int(i): cj[s:e] for i, s, e in zip(row_ids, row_starts, row_ends)}

    flag = np.zeros(max(n, int(g.max()) + 1), dtype=bool)
    for b in np.unique(row_ids // B):
        bs = int(b) * B
        g0 = g.copy()
        frozen = []
        for i in range(bs, bs + B):
            J = row_j.get(i)
            if J is None:
                continue
            J = J[g0[J] != g0[i]]
            if J.size:
                frozen.append((i, J))
        for i, J in frozen:
            mg = np.unique(g[J])
            flag[mg] = True
            sel = flag[g]
            g[sel] = g[i]
            flag[mg] = False
    return g


def kernel(V, group_ids, cos_threshold, batch_size):
    V32 = np.ascontiguousarray(np.asarray(V, dtype=np.float32))
    g = np.asarray(group_ids, dtype=np.int32).copy()
    thr = float(np.asarray(cos_threshold).reshape(-1)[0])
    B = int(np.asarray(batch_size))

    ci, cj = _device_candidate_edges(V32, thr)
    ci, cj = _exact_edges(V32, ci, cj, thr, B)
    g = _merge_replay(g, ci, cj, B)
    return g.astype(np.int32)
